# revision 55
# baseline (speedup 1.0000x reference)
"""Transformer encoder layer (nn_Encoder) on 8 TRN2 NeuronCores.

Strategy: data-parallel over batch — B=8, one batch element per core, weights
replicated, no collectives. Per core a single Bass/Tile kernel computes the
whole layer; large matmuls run in fp32r (full PE rate); attention softmax
operands (q/k/v65/exp) and the whole FFN run in bf16 (same PE stream rate,
FWL weight loads, half the SBUF/DMA), output returned bf16 and upcast on host.

Layout: attention runs in the "transposed domain" ([feature, tokens]); softmax
over tokens-on-partitions is handled by appending a ones-column to V (denom
lands in the ctx matmul's extra output row, M=65). Per (pair, slice) the two
denominator rows are staged into a [2, NS] tile, inverted, broadcast across
partitions with a K=2 selection matmul and applied in one full-width multiply,
deferred half a pair to keep PE fed. Wo/FFN products land in the natural
domain where both LayerNorms reduce along the free dim (scalar-engine accum);
LN tail adds (+be, +b2) run on the otherwise-idle GpSimd engine. h^T bridges
to FFN1 via 64 PE transposes. FFN runs over the full sequence: W1 is streamed
once (bf16), W2 is SBUF-resident (bf16, loaded during the Wo phase), FFN2 is
grouped per-si so each LN2 overlaps the next group's matmuls instead of
forming a serial tail.

Self-contained: hardcodes B=8, S=1024, D=1024, H=16, FF=2048, 8 cores.
"""
import math
import numpy as np
from contextlib import ExitStack

import concourse.bass as bass
import concourse.tile as tile
from concourse import bacc, mybir
from concourse import bass_utils
from concourse.masks import make_identity

B = 8
S = 1024
D = 1024
H = 16
FF = 2048
P = 128
HD = 64
EPS = 1e-5
f32 = mybir.dt.float32
f32r = mybir.dt.float32r
bf16 = mybir.dt.bfloat16
AF = mybir.ActivationFunctionType
ALU = mybir.AluOpType
AX = mybir.AxisListType

NP_ = H // 2          # head pairs
ST = S // P           # token tiles
DT = D // P
FT = FF // P
NS = 512              # token slice width (matmul free dim)
SL = S // NS
ND = 512              # feature slice width
DL = D // ND


def _layer_norm(nc, pool, v, out, g_b, be_b, si, pfx, eps_c):
    """LayerNorm over the free dim of v [128, D] -> out = norm(v)*g + be.
    Sums + the (v-mu)*rstd pass on the Scalar engine (accum_out / Identity
    with per-partition scale+bias); DVE does small ops + 2 full passes."""
    scr = pool.tile([P, D], f32, name=f"{pfx}scr{si}", tag=f"{pfx}scr", bufs=3)
    st = pool.tile([P, 8], f32, name=f"{pfx}st{si}", tag=f"{pfx}st", bufs=4)
    s1 = st[:, 0:1]; s2 = st[:, 1:2]; mu = st[:, 2:3]; var = st[:, 3:4]
    sd = st[:, 4:5]; rstd = st[:, 5:6]; nm = st[:, 6:7]
    nc.scalar.activation(scr[:], v[:], AF.Copy, accum_out=s1)
    nc.scalar.activation(scr[:], v[:], AF.Square, accum_out=s2)
    nc.vector.tensor_scalar_mul(mu, s1, 1.0 / D)
    nc.vector.tensor_mul(sd, mu, mu)
    nc.vector.tensor_scalar(out=var, in0=s2, scalar1=1.0 / D, scalar2=sd,
                            op0=ALU.mult, op1=ALU.subtract)
    nc.scalar.activation(sd, var, AF.Sqrt, bias=eps_c)
    nc.vector.reciprocal(rstd, sd)
    nc.vector.tensor_scalar(out=nm, in0=mu, scalar1=rstd, scalar2=-1.0,
                            op0=ALU.mult, op1=ALU.mult)
    nc.scalar.activation(scr[:], v[:], AF.Identity, bias=nm, scale=rstd)
    # v is dead after the Identity pass; reuse it so the multiply is not
    # in-place (in-place DVE tensor ops run ~2x slower)
    nc.vector.tensor_mul(v[:], scr[:], g_b[:])
    nc.vector.tensor_add(out[:], v[:], be_b[:])


def build_encoder(num_devices=8):
    scale = 1.0 / math.sqrt(HD)
    nc = bacc.Bacc("TRN2", target_bir_lowering=False, debug=False,
                   enable_asserts=True, num_devices=num_devices)

    dram = lambda n, sh, dt: nc.dram_tensor(n, sh, dt, kind="ExternalInput").ap()
    xT_d = dram("xT", [D, S], bf16)
    xbo_d = dram("xbo", [S, D], f32)          # x + bo, folded on host
    wq_d = dram("Wq", [NP_, P, DT * P], bf16)  # [pair, dp, dt*q] contiguous
    wk_d = dram("Wk", [NP_, P, DT * P], bf16)
    wv_d = dram("Wv", [D, D], bf16)
    wo_d = dram("Wo", [D, D], bf16)
    w1_d = dram("W1", [FT, P, DT * P], bf16)   # [f, dp, dt*q] contiguous
    w2_d = dram("W2", [FF, D], bf16)
    sel2_d = dram("sel2", [2, P], f32r)
    bqc_d = dram("bqc", [P, NP_], f32)
    bkc_d = dram("bkc", [P, NP_], f32)
    b1c_d = dram("b1c", [P, FT], f32)
    bv_d = dram("bv", [D], f32)
    b2_d = dram("b2", [D], f32)
    g1_d = dram("g1", [D], f32)
    be1_d = dram("be1", [D], f32)
    g2_d = dram("g2", [D], f32)
    be2_d = dram("be2", [D], f32)
    out_d = nc.dram_tensor("out", [S, D], bf16, kind="ExternalOutput").ap()

    with tile.TileContext(nc) as tc, ExitStack() as octx:
        const = octx.enter_context(tc.tile_pool(name="const", bufs=1))
        identity = const.tile([P, P], bf16, name="identity")
        make_identity(nc, identity)
        bqc = const.tile([P, NP_], f32, name="bqc")
        nc.sync.dma_start(bqc[:], bqc_d)
        bkc = const.tile([P, NP_], f32, name="bkc")
        nc.sync.dma_start(bkc[:], bkc_d)
        b1c = const.tile([P, FT], f32, name="b1c")
        nc.sync.dma_start(b1c[:], b1c_d)
        # sel2[h, m] = 1.0 iff m // HD == h  (K=2 denominator broadcast)
        sel2 = const.tile([2, P], f32r, name="sel2")
        nc.sync.dma_start(sel2[:], sel2_d)
        eps_t = const.tile([P, 1], f32, name="eps_c")
        nc.gpsimd.memset(eps_t[:], EPS)
        eps_c = eps_t[:, 0:1]

        def bcast_row(pool, name, src_row, width):
            r = pool.tile([1, width], f32, name=f"{name}_r", tag="bcr", bufs=1)
            nc.sync.dma_start(r[:], src_row[None, :])
            b = pool.tile([P, width], f32, name=f"{name}_b", tag=f"{name}_b")
            nc.gpsimd.partition_broadcast(b[:], r[:])
            return b

        # ctxT pool (attention -> Wo; stays open to end of build)
        pCtx = octx.enter_context(tc.tile_pool(name="pCtx", bufs=1))
        # Wo weights prefetched during attention so the Wo phase starts hot
        # (DMAs are emitted after the attention-critical xt/wq loads)
        pPre = octx.enter_context(tc.tile_pool(name="pPre", bufs=1))
        wo = [pPre.tile([P, D], bf16, name=f"wo{p}", tag="wo", bufs=NP_)
              for p in range(NP_)]

        # ---------------- attention scope ----------------
        with tc.tile_pool(name="pA", bufs=1) as pA, \
             tc.tile_pool(name="psA", bufs=1, space="PSUM") as psA:

            # pair-0 Q/K weights first so QK(0) matmuls start ASAP
            wq0 = pA.tile([P, DT * P], bf16, name="wq0", tag="wq", bufs=2)
            nc.sync.dma_start(wq0[:], wq_d[0])
            wk0 = pA.tile([P, DT * P], bf16, name="wk0", tag="wk", bufs=2)
            nc.sync.dma_start(wk0[:], wk_d[0])

            # x^T tiles; first-needed halves first
            xt = [pA.tile([P, S], bf16, name=f"xt{d}", tag="xt", bufs=DT)
                  for d in range(DT)]
            for d in range(4):
                nc.sync.dma_start(xt[d][:, 0:NS], xT_d[d * P:(d + 1) * P, 0:NS])
            for d in range(4, DT):
                nc.sync.dma_start(xt[d][:, 0:NS], xT_d[d * P:(d + 1) * P, 0:NS])
            for d in range(DT):
                nc.sync.dma_start(xt[d][:, NS:S], xT_d[d * P:(d + 1) * P, NS:S])

            bv_b = bcast_row(pA, "bv", bv_d, D)
            for p_ in range(NP_):
                nc.sync.dma_start(wo[p_][:], wo_d[p_ * P:(p_ + 1) * P, :])

            # V65 tiles padded to 128 cols/head (full-width LDWEIGHTS can be
            # pulled ahead of in-flight matmuls; 65-col ones serialize).
            # Cols 0-63 = V, col 64 = ones (softmax denominator), 65-127 = 0.
            VC = 128
            v65 = []
            for t in range(ST):
                v = pA.tile([P, H * VC], bf16, name=f"v65_{t}", tag="v65",
                            bufs=ST)
                nc.gpsimd.memset(v[:], 0.0)
                nc.gpsimd.memset(
                    v.rearrange("p (h c) -> p h c", c=VC)[:, :, 64:65], 1.0)
                v65.append(v)

            pExp_cm = tc.tile_pool(name="pExp", bufs=1)
            pExp = pExp_cm.__enter__()

            # ---- V projection (wv pool; chunks emitted inside pair 0) ----
            pV_cm = tc.tile_pool(name="pV", bufs=1)
            pV = pV_cm.__enter__()
            wv = []
            for d in range(DT):
                t = pV.tile([P, D], bf16, name=f"wv{d}", tag="wv", bufs=DT)
                nc.sync.dma_start(t[:], wv_d[d * P:(d + 1) * P, :])
                wv.append(t)
            hpn = ND // HD
            v_state = {}

            def emit_v_chunk(hc):
                """Half-chunk hc of the V projection (chain = hc//2)."""
                chain = hc // 2
                part = hc % 2
                t, n = chain // DL, chain % DL
                if part == 0:
                    v_state[chain] = psA.tile(
                        [P, ND], f32, name=f"vps{t}_{n}", tag="vqk", bufs=2)
                ps = v_state[chain]
                for d in range(4 * part, 4 * part + 4):
                    nc.tensor.matmul(
                        ps[:], xt[d][:, t * P:(t + 1) * P],
                        wv[d][:, n * ND:(n + 1) * ND],
                        start=(d == 0), stop=(d == DT - 1))
                if part == 1:
                    dst = v65[t].rearrange("p (h c) -> p h c", c=VC)[
                        :, n * hpn:(n + 1) * hpn, 0:64]
                    srcv = ps[:].rearrange("p (h k) -> p h k", k=HD)
                    bvs = bv_b[:, n * ND:(n + 1) * ND].rearrange(
                        "p (h k) -> p h k", k=HD)
                    nc.vector.tensor_add(dst, srcv, bvs)

            # ---- attention per head pair ----
            ctxT = [pCtx.tile([P, S], bf16, name=f"ctxT{p}", tag="ctxT",
                              bufs=NP_) for p in range(NP_)]

            def emit_qk_chain_part(p, chain, part, state):
                """Emit 4 of the 8 accumulation matmuls of QK chain
                (chain: 0..3 = Q-sl0, Q-sl1, K-sl0, K-sl1) for pair p."""
                wt, bc, dst = state["ops"][chain // 2]
                sl = chain % 2
                if part == 0:
                    state[chain] = psA.tile(
                        [P, NS], f32, name=f"qk{p}_{chain}", tag="vqk", bufs=2)
                ps = state[chain]
                for d in range(4 * part, 4 * part + 4):
                    nc.tensor.matmul(
                        ps[:], wt[:, d * P:(d + 1) * P],
                        xt[d][:, sl * NS:(sl + 1) * NS],
                        start=(d == 0), stop=(d == DT - 1))
                if part == 1:
                    nc.vector.tensor_scalar(
                        out=dst[:, sl * NS:(sl + 1) * NS], in0=ps[:],
                        scalar1=bc[:, p:p + 1], scalar2=None, op0=ALU.add)

            def make_qk_state(p):
                if p == 0:
                    wqt, wkt = wq0, wk0
                else:
                    wqt = pA.tile([P, DT * P], bf16, name=f"wq{p}", tag="wq",
                                  bufs=2)
                    nc.sync.dma_start(wqt[:], wq_d[p])
                    wkt = pA.tile([P, DT * P], bf16, name=f"wk{p}", tag="wk",
                                  bufs=2)
                    nc.sync.dma_start(wkt[:], wk_d[p])
                qt = pA.tile([P, S], bf16, name=f"qt{p}", tag="qt", bufs=2)
                kt = pA.tile([P, S], bf16, name=f"kt{p}", tag="kt", bufs=2)
                return {"ops": ((wqt, bqc, qt), (wkt, bkc, kt)),
                        "qt": qt, "kt": kt}

            LAG = 2
            qk_state = make_qk_state(0)
            for chain in range(4):
                for part in range(2):
                    emit_qk_chain_part(0, chain, part, qk_state)

            # deferred softmax-normalize queue: entries (p, sl, ctxU, den2)
            norm_q = []

            def emit_normalize():
                if not norm_q:
                    return
                p, sl, ctxU, den2 = norm_q.pop(0)
                den2f = pA.tile([2, NS], f32, name=f"d2f{p}_{sl}",
                                tag="den2f", bufs=3)
                nc.vector.reciprocal_approx_fast(den2f[:], den2[:])
                den2r = pA.tile([2, NS], f32r, name=f"d2r{p}_{sl}",
                                tag="den2r", bufs=3)
                nc.vector.tensor_copy(den2r[:], den2f[:])
                rcb = psA.tile([P, NS], f32, name=f"rcb{p}_{sl}",
                               tag="vqk", bufs=2)
                nc.tensor.matmul(rcb[:], sel2[:], den2r[:],
                                 start=True, stop=True)
                nc.vector.tensor_mul(
                    ctxT[p][:, sl * NS:(sl + 1) * NS],
                    ctxU[:, sl * NS:(sl + 1) * NS], rcb[:])

            pending = None
            for p in range(NP_):
                qt, kt = qk_state["qt"], qk_state["kt"]
                next_state = make_qk_state(p + 1) if p + 1 < NP_ else None

                ctxU = pA.tile([P, S], f32, name=f"ctxU{p}", tag="ctxU",
                               bufs=2)

                def emit_scores(sl, t, expt):
                    ps = psA.tile([P, 2 * NS], f32, name=f"sc{t}_{sl}",
                                  tag="sc", bufs=2)
                    for h in range(2):
                        nc.tensor.matmul(
                            ps[:, h * NS:(h + 1) * NS],
                            kt[h * HD:(h + 1) * HD, t * P:(t + 1) * P],
                            qt[h * HD:(h + 1) * HD, sl * NS:(sl + 1) * NS],
                            start=True, stop=True,
                            tile_position=(h * HD, 0))
                    e = pExp.tile([P, 2 * NS], bf16, name=f"e{t}_{sl}",
                                  tag="exp", bufs=3)
                    nc.scalar.activation(e[:], ps[:], AF.Exp, scale=scale)
                    expt[t] = e

                def emit_ctx(sl, tt, cps, expt):
                    for h in range(2):
                        hc = (2 * p + h) * VC
                        nc.tensor.matmul(
                            cps[h][:], v65[tt][:, hc:hc + VC],
                            expt[tt][:, h * NS:(h + 1) * NS],
                            start=(tt == 0), stop=(tt == ST - 1))

                def emit_evict(sl, cps):
                    den2 = pA.tile([2, NS], f32, name=f"den{p}_{sl}",
                                   tag="den2", bufs=3)
                    for h in range(2):
                        ps = cps[h]
                        stage = pA.tile([65, NS], f32, name=f"stg{h}{sl}",
                                        tag="rc", bufs=2)
                        nc.vector.tensor_copy(stage[64:65, :], ps[64:65, :])
                        nc.sync.dma_start(den2[h:h + 1, :], stage[64:65, :])
                        if h == 0:
                            nc.vector.tensor_copy(
                                ctxU[0:HD, sl * NS:(sl + 1) * NS],
                                ps[0:HD, :])
                        else:
                            tmp = pA.tile([HD, NS], f32, name=f"ctmp{sl}",
                                          tag="ctmp", bufs=2)
                            nc.vector.tensor_copy(tmp[:], ps[0:HD, :])
                            nc.sync.dma_start(
                                ctxU[HD:P, sl * NS:(sl + 1) * NS], tmp[:])
                    norm_q.append((p, sl, ctxU, den2))

                expt0 = {}
                cps0 = [psA.tile([P, NS], f32, name=f"cps{h}_0", tag="ctx",
                                 bufs=2) for h in range(2)]
                expt1 = {}
                cps1 = [psA.tile([P, NS], f32, name=f"cps{h}_1", tag="ctx",
                                 bufs=2) for h in range(2)]
                if p == 0:
                    # A: scores(sl0) + the whole V projection interleaved
                    for t in range(ST):
                        emit_scores(0, t, expt0)
                        for hc in range(4 * t, 4 * t + 4):
                            emit_v_chunk(hc)
                    # B: scores(sl1) + lagged ctx(sl0)
                    for t in range(ST + LAG):
                        if t < ST:
                            emit_scores(1, t, expt1)
                        if t >= LAG:
                            emit_ctx(0, t - LAG, cps0, expt0)
                    emit_evict(0, cps0)
                    # C: ctx(sl1) + QK(1) chunks
                    for t in range(ST):
                        emit_ctx(1, t, cps1, expt1)
                        if t == 2:
                            emit_normalize()          # (0, sl0)
                        if next_state is not None:
                            emit_qk_chain_part(p + 1, t // 2, t % 2,
                                               next_state)
                    emit_evict(1, cps1)
                    pV_cm.__exit__(None, None, None)
                else:
                    # A: scores(sl0) + QK(p+1) chunks 0-3 + lagged ctx(sl0)
                    for t in range(ST + LAG):
                        if t < ST:
                            emit_scores(0, t, expt0)
                            if next_state is not None and t < 4:
                                emit_qk_chain_part(p + 1, t // 2, t % 2,
                                                   next_state)
                        if t == 2:
                            emit_normalize()          # (p-1, sl1)
                        if t >= LAG:
                            emit_ctx(0, t - LAG, cps0, expt0)
                    emit_evict(0, cps0)
                    # B: scores(sl1) + QK(p+1) chunks 4-7 + lagged ctx(sl1)
                    for t in range(ST + LAG):
                        if t < ST:
                            emit_scores(1, t, expt1)
                            if next_state is not None and t < 4:
                                emit_qk_chain_part(p + 1, (t + 4) // 2,
                                                   t % 2, next_state)
                        if t == 2:
                            emit_normalize()          # (p, sl0)
                        if t >= LAG:
                            emit_ctx(1, t - LAG, cps1, expt1)
                    emit_evict(1, cps1)
                qk_state = next_state
            # pair 7, sl0 + sl1 normalize straight away (Wo accumulates
            # pair 7 last, giving the den chain ~3.5us of PE slack)
            emit_normalize()
            emit_normalize()
            pExp_cm.__exit__(None, None, None)

        # ---------------- resident pools (open after attention) ----------
        pH = octx.enter_context(tc.tile_pool(name="pH", bufs=1))
        ht = [pH.tile([P, S], bf16, name=f"ht{d}", tag="ht", bufs=DT)
              for d in range(DT)]
        h_nat = [pH.tile([P, D], bf16, name=f"hn{si}", tag="hn", bufs=ST)
                 for si in range(ST)]
        pW2 = octx.enter_context(tc.tile_pool(name="pW2", bufs=1))
        w2r = [pW2.tile([P, D], bf16, name=f"w2r{f}", tag="w2r", bufs=FT)
               for f in range(FT)]
        w1_tiles = []

        # ---------------- Wo + LN1 scope ----------------
        with tc.tile_pool(name="pWo", bufs=1) as pWo, \
             tc.tile_pool(name="psW", bufs=1, space="PSUM") as psW:
            g1_b = bcast_row(pWo, "g1", g1_d, D)
            be1_b = bcast_row(pWo, "be1", be1_d, D)
            # prefetch the first W1 stream tiles so FFN1 starts immediately
            for f in range(3):
                w1t = pW2.tile([P, DT * P], bf16, name=f"w1_{f}",
                               tag="w1", bufs=3)
                nc.sync.dma_start(w1t[:], w1_d[f])
                w1_tiles.append(w1t)

            def emit_tp(si):
                """h^T transposes for si (lagged 2 si so the LN1 chain
                latency never blocks the in-order PE queue); copybacks
                alternate DVE/ACT; then fold b2 into the residual."""
                hn = h_nat[si]
                for dd in range(DT):
                    ps = psW.tile([P, P], bf16, name=f"tp{si}_{dd}", tag="tp",
                                  bufs=4)
                    nc.tensor.transpose(
                        ps[:], hn[:, dd * P:(dd + 1) * P], identity[:])
                    dst = ht[dd][:, si * P:(si + 1) * P]
                    if dd % 2 == 0:
                        nc.vector.tensor_copy(dst, ps[:])
                    else:
                        nc.scalar.copy(dst, ps[:])

            for si in range(ST):
                xn = pWo.tile([P, D], f32, name=f"xn{si}", tag="xn", bufs=3)
                nc.sync.dma_start(xn[:], xbo_d[si * P:(si + 1) * P, :])
                pss = [psW.tile([P, ND], f32, name=f"c{si}_{n}", tag="c",
                                bufs=4) for n in range(DL)]
                for p in range(NP_):
                    for n in range(DL):
                        nc.tensor.matmul(
                            pss[n][:], ctxT[p][:, si * P:(si + 1) * P],
                            wo[p][:, n * ND:(n + 1) * ND],
                            start=(p == 0), stop=(p == NP_ - 1))
                v = pWo.tile([P, D], f32, name=f"v{si}", tag="v", bufs=3)
                for n in range(DL):
                    nc.vector.tensor_add(v[:, n * ND:(n + 1) * ND], pss[n][:],
                                         xn[:, n * ND:(n + 1) * ND])
                hn = h_nat[si]
                _layer_norm(nc, pWo, v, hn, g1_b, be1_b, si, "ln1", eps_c)
                if si >= 3:
                    emit_tp(si - 3)
            for si in range(ST - 3, ST):
                emit_tp(si)

        # ---------------- FFN + LN2 scope (full sequence) ----------------
        with tc.tile_pool(name="pF", bufs=1) as pF:
            g2_b = bcast_row(pF, "g2", g2_d, D)
            be2_b = bcast_row(pF, "be2", be2_d, D)
            b2_b = bcast_row(pF, "b2", b2_d, D)

            # FFN1 + relu over full S; W1 streamed once (bf16)
            ut = []
            with tc.tile_pool(name="psU", bufs=1, space="PSUM") as psU:
                for f in range(FT):
                    if f < 3:
                        w1t = w1_tiles[f]
                    else:
                        w1t = pW2.tile([P, DT * P], bf16, name=f"w1_{f}",
                                       tag="w1", bufs=3)
                        nc.sync.dma_start(w1t[:], w1_d[f])
                    # W2 resident load rides along, spread over FFN1
                    nc.sync.dma_start(w2r[f][:], w2_d[f * P:(f + 1) * P, :])
                    u = pF.tile([P, S], bf16, name=f"ut{f}", tag="ut",
                                bufs=FT)
                    pss = [psU.tile([P, NS], f32, name=f"u{f}_{sh}", tag="u",
                                    bufs=4) for sh in range(SL)]
                    for d in range(DT):
                        for sh in range(SL):
                            nc.tensor.matmul(
                                pss[sh][:], w1t[:, d * P:(d + 1) * P],
                                ht[d][:, sh * NS:(sh + 1) * NS],
                                start=(d == 0), stop=(d == DT - 1))
                    for sh in range(SL):
                        nc.scalar.activation(
                            u[:, sh * NS:(sh + 1) * NS], pss[sh][:], AF.Relu,
                            bias=b1c[:, f:f + 1])
                    ut.append(u)
                    # fold the FFN2 output bias into the residual on the
                    # otherwise-idle DVE (transposes consumed clean h already)
                    if f < ST:
                        nc.vector.tensor_add(h_nat[f][:], h_nat[f][:],
                                             b2_b[:])

            # FFN2 + LN2 + out: per-si groups; W2 resident so the f-loop
            # restreams freely; LN2 of si overlaps matmuls of si+1
            with tc.tile_pool(name="psY", bufs=1, space="PSUM") as psY:
                for si in range(ST):
                    pss = [psY.tile([P, ND], f32, name=f"y{si}_{n}", tag="y",
                                    bufs=8) for n in range(DL)]
                    for f in range(FT):
                        for n in range(DL):
                            nc.tensor.matmul(
                                pss[n][:],
                                ut[f][:, si * P:(si + 1) * P],
                                w2r[f][:, n * ND:(n + 1) * ND],
                                start=(f == 0), stop=(f == FT - 1))
                    v = pF.tile([P, D], f32, name=f"v2_{si}", tag="v2",
                                bufs=3)
                    for n in range(DL):
                        nc.vector.tensor_add(
                            v[:, n * ND:(n + 1) * ND], pss[n][:],
                            h_nat[si][:, n * ND:(n + 1) * ND])
                    o = pF.tile([P, D], bf16, name=f"o{si}", tag="o",
                                bufs=3)
                    _layer_norm(nc, pF, v, o, g2_b, be2_b, si, "ln2", eps_c)
                    nc.sync.dma_start(out_d[si * P:(si + 1) * P, :], o[:])

    nc.compile()
    return nc


from ml_dtypes import bfloat16 as _np_bf16


def pack_core_inputs(x_b, shared):
    """Per-core input map: batch element x_b + shared (prepacked) weights."""
    m = dict(shared)
    bo_row = m.pop("_bo_row")
    x_b = np.asarray(x_b, dtype=np.float32)
    m["xbo"] = np.ascontiguousarray(x_b + bo_row)
    m["xT"] = np.ascontiguousarray(x_b.T).astype(_np_bf16)
    return m


def pack_shared(Wq, bq, Wk, bk, Wv, bv, Wo, bo, ln1_g, ln1_b, W1, b1, W2, b2,
                ln2_g, ln2_b):
    """Host-side layout packing of the replicated weights (pure layout)."""
    from ml_dtypes import bfloat16 as np_bf16
    f = np.float32
    Wq = np.asarray(Wq, dtype=f); Wk = np.asarray(Wk, dtype=f)
    Wv = np.asarray(Wv, dtype=f)
    # [dt, dp, h2, q] -> [h2, dp, dt, q] contiguous
    pack_qk = lambda W: np.ascontiguousarray(
        W.reshape(D, H * HD).reshape(DT, P, NP_, P).transpose(2, 1, 0, 3)
        .reshape(NP_, P, DT * P))
    W1p = (np.asarray(W1, dtype=f).reshape(DT, P, FT, P)
           .transpose(2, 1, 0, 3).reshape(FT, P, DT * P))
    sel2 = np.zeros((2, P), dtype=f)
    sel2[0, 0:HD] = 1.0
    sel2[1, HD:P] = 1.0
    return {
        "sel2": sel2,
        "Wq": pack_qk(Wq).astype(np_bf16), "Wk": pack_qk(Wk).astype(np_bf16),
        "Wv": np.ascontiguousarray(Wv.reshape(D, D)).astype(np_bf16),
        "Wo": np.ascontiguousarray(Wo, dtype=f).astype(np_bf16),
        "W1": np.ascontiguousarray(W1p).astype(np_bf16),
        "W2": np.ascontiguousarray(W2, dtype=f).astype(np_bf16),
        "bqc": np.ascontiguousarray(np.asarray(bq, f).reshape(NP_, P).T),
        "bkc": np.ascontiguousarray(np.asarray(bk, f).reshape(NP_, P).T),
        "b1c": np.ascontiguousarray(np.asarray(b1, f).reshape(FT, P).T),
        "bv": np.ascontiguousarray(np.asarray(bv, f).reshape(D)),
        "b2": np.ascontiguousarray(b2, dtype=f),
        "g1": np.ascontiguousarray(ln1_g, dtype=f),
        "be1": np.ascontiguousarray(ln1_b, dtype=f),
        "g2": np.ascontiguousarray(ln2_g, dtype=f),
        "be2": np.ascontiguousarray(ln2_b, dtype=f),
        "_bo_row": np.asarray(bo, dtype=f).reshape(1, D),
    }


_NC_CACHE = {}


def get_nc():
    if "nc" not in _NC_CACHE:
        _NC_CACHE["nc"] = build_encoder(num_devices=8)
    return _NC_CACHE["nc"]


def kernel(x, Wq, bq, Wk, bk, Wv, bv, Wo, bo, ln1_g, ln1_b, W1, b1, W2, b2,
           ln2_g, ln2_b):
    x = np.asarray(x)
    assert x.shape == (B, S, D)
    shared = pack_shared(Wq, bq, Wk, bk, Wv, bv, Wo, bo, ln1_g, ln1_b,
                         W1, b1, W2, b2, ln2_g, ln2_b)
    in_maps = [pack_core_inputs(x[b], shared) for b in range(B)]
    nc = get_nc()
    res = bass_utils.run_bass_kernel_spmd(
        nc, in_maps, core_ids=list(range(B)), trace=False)
    return np.stack([np.asarray(res.results[b]["out"], dtype=np.float32)
                     for b in range(B)], axis=0)


# revision 56
# speedup vs baseline: 1.0282x; 1.0282x over previous
"""Transformer encoder layer (nn_Encoder) on 8 TRN2 NeuronCores.

Strategy: data-parallel over batch — B=8, one batch element per core, weights
replicated, no collectives. Per core a single Bass/Tile kernel computes the
whole layer; large matmuls run in fp32r (full PE rate); attention softmax
operands (q/k/v65/exp) and the whole FFN run in bf16 (same PE stream rate,
FWL weight loads, half the SBUF/DMA), output returned bf16 and upcast on host.

Layout: attention runs in the "transposed domain" ([feature, tokens]); softmax
over tokens-on-partitions is handled by appending a ones-column to V (denom
lands in the ctx matmul's extra output row, M=65). Per (pair, slice) the two
denominator rows are staged into a [2, NS] tile, inverted, broadcast across
partitions with a K=2 selection matmul and applied in one full-width multiply,
deferred half a pair to keep PE fed. Wo/FFN products land in the natural
domain where both LayerNorms reduce along the free dim (scalar-engine accum);
LN tail adds (+be, +b2) run on the otherwise-idle GpSimd engine. h^T bridges
to FFN1 via 64 PE transposes. FFN runs over the full sequence: W1 is streamed
once (bf16), W2 is SBUF-resident (bf16, loaded during the Wo phase), FFN2 is
grouped per-si so each LN2 overlaps the next group's matmuls instead of
forming a serial tail.

Self-contained: hardcodes B=8, S=1024, D=1024, H=16, FF=2048, 8 cores.
"""
import math
import numpy as np
from contextlib import ExitStack

import concourse.bass as bass
import concourse.tile as tile
from concourse import bacc, mybir
from concourse import bass_utils
from concourse.masks import make_identity

B = 8
S = 1024
D = 1024
H = 16
FF = 2048
P = 128
HD = 64
EPS = 1e-5
f32 = mybir.dt.float32
f32r = mybir.dt.float32r
bf16 = mybir.dt.bfloat16
AF = mybir.ActivationFunctionType
ALU = mybir.AluOpType
AX = mybir.AxisListType

NP_ = H // 2          # head pairs
ST = S // P           # token tiles
DT = D // P
FT = FF // P
NS = 512              # token slice width (matmul free dim)
SL = S // NS
ND = 512              # feature slice width
DL = D // ND


def _layer_norm(nc, pool, v, out, g_b, be_b, si, pfx):
    """LayerNorm over the free dim of v [128, D] -> out = norm(v)*g + be.
    Sums + the (v-mu)*rstd pass on the Scalar engine (accum_out / Identity
    with per-partition scale+bias); DVE does small ops + 2 full passes."""
    scr = pool.tile([P, D], f32, name=f"{pfx}scr{si}", tag=f"{pfx}scr", bufs=3)
    st = pool.tile([P, 8], f32, name=f"{pfx}st{si}", tag=f"{pfx}st", bufs=4)
    s1 = st[:, 0:1]; s2 = st[:, 1:2]; mu = st[:, 2:3]; var = st[:, 3:4]
    sd = st[:, 4:5]; rstd = st[:, 5:6]; nm = st[:, 6:7]
    nc.scalar.activation(scr[:], v[:], AF.Copy, accum_out=s1)
    nc.scalar.activation(scr[:], v[:], AF.Square, accum_out=s2)
    nc.vector.tensor_scalar_mul(mu, s1, 1.0 / D)
    nc.vector.tensor_scalar_mul(var, s2, 1.0 / D)
    nc.vector.tensor_mul(sd, mu, mu)
    nc.vector.tensor_sub(var, var, sd)
    nc.vector.tensor_scalar_add(var, var, EPS)
    nc.scalar.sqrt(sd, var)
    nc.vector.reciprocal(rstd, sd)
    nc.vector.tensor_mul(nm, mu, rstd)
    nc.vector.tensor_scalar_mul(nm, nm, -1.0)
    nc.scalar.activation(scr[:], v[:], AF.Identity, bias=nm, scale=rstd)
    # v is dead after the Identity pass; reuse it so the multiply is not
    # in-place (in-place DVE tensor ops run ~2x slower)
    nc.vector.tensor_mul(v[:], scr[:], g_b[:])
    nc.vector.tensor_add(out[:], v[:], be_b[:])


def build_encoder(num_devices=8):
    scale = 1.0 / math.sqrt(HD)
    nc = bacc.Bacc("TRN2", target_bir_lowering=False, debug=False,
                   enable_asserts=True, num_devices=num_devices)

    dram = lambda n, sh, dt: nc.dram_tensor(n, sh, dt, kind="ExternalInput").ap()
    xT_d = dram("xT", [D, S], bf16)
    xbo_d = dram("xbo", [S, D], f32)          # x + bo, folded on host
    wq_d = dram("Wq", [NP_, P, DT * P], bf16)  # [pair, dp, dt*q] contiguous
    wk_d = dram("Wk", [NP_, P, DT * P], bf16)
    wv_d = dram("Wv", [D, D], bf16)
    wo_d = dram("Wo", [D, D], bf16)
    w1_d = dram("W1", [FT, P, DT * P], bf16)   # [f, dp, dt*q] contiguous
    w2_d = dram("W2", [FF, D], bf16)
    sel2_d = dram("sel2", [2, P], f32r)
    bqc_d = dram("bqc", [P, NP_], f32)
    bkc_d = dram("bkc", [P, NP_], f32)
    b1c_d = dram("b1c", [P, FT], f32)
    bv_d = dram("bv", [D], f32)
    b2_d = dram("b2", [D], f32)
    g1_d = dram("g1", [D], f32)
    be1_d = dram("be1", [D], f32)
    g2_d = dram("g2", [D], f32)
    be2_d = dram("be2", [D], f32)
    out_d = nc.dram_tensor("out", [S, D], bf16, kind="ExternalOutput").ap()

    with tile.TileContext(nc) as tc, ExitStack() as octx:
        const = octx.enter_context(tc.tile_pool(name="const", bufs=1))
        identity = const.tile([P, P], bf16, name="identity")
        make_identity(nc, identity)
        bqc = const.tile([P, NP_], f32, name="bqc")
        nc.sync.dma_start(bqc[:], bqc_d)
        bkc = const.tile([P, NP_], f32, name="bkc")
        nc.sync.dma_start(bkc[:], bkc_d)
        b1c = const.tile([P, FT], f32, name="b1c")
        nc.sync.dma_start(b1c[:], b1c_d)
        # sel2[h, m] = 1.0 iff m // HD == h  (K=2 denominator broadcast)
        sel2 = const.tile([2, P], f32r, name="sel2")
        nc.sync.dma_start(sel2[:], sel2_d)

        def bcast_row(pool, name, src_row, width):
            r = pool.tile([1, width], f32, name=f"{name}_r", tag="bcr", bufs=1)
            nc.sync.dma_start(r[:], src_row[None, :])
            b = pool.tile([P, width], f32, name=f"{name}_b", tag=f"{name}_b")
            nc.gpsimd.partition_broadcast(b[:], r[:])
            return b

        # ctxT pool (attention -> Wo; stays open to end of build)
        pCtx = octx.enter_context(tc.tile_pool(name="pCtx", bufs=1))
        # Wo weights prefetched during attention so the Wo phase starts hot
        # (DMAs are emitted after the attention-critical xt/wq loads)
        pPre = octx.enter_context(tc.tile_pool(name="pPre", bufs=1))
        wo = [pPre.tile([P, D], bf16, name=f"wo{p}", tag="wo", bufs=NP_)
              for p in range(NP_)]

        # ---------------- attention scope ----------------
        with tc.tile_pool(name="pA", bufs=1) as pA, \
             tc.tile_pool(name="psA", bufs=1, space="PSUM") as psA:

            # pair-0 Q/K weights first so QK(0) matmuls start ASAP
            wq0 = pA.tile([P, DT * P], bf16, name="wq0", tag="wq", bufs=2)
            nc.sync.dma_start(wq0[:], wq_d[0])
            wk0 = pA.tile([P, DT * P], bf16, name="wk0", tag="wk", bufs=2)
            nc.sync.dma_start(wk0[:], wk_d[0])

            # x^T tiles; first-needed halves first
            xt = [pA.tile([P, S], bf16, name=f"xt{d}", tag="xt", bufs=DT)
                  for d in range(DT)]
            for d in range(4):
                nc.sync.dma_start(xt[d][:, 0:NS], xT_d[d * P:(d + 1) * P, 0:NS])
            for d in range(4, DT):
                nc.sync.dma_start(xt[d][:, 0:NS], xT_d[d * P:(d + 1) * P, 0:NS])
            for d in range(DT):
                nc.sync.dma_start(xt[d][:, NS:S], xT_d[d * P:(d + 1) * P, NS:S])

            bv_b = bcast_row(pA, "bv", bv_d, D)
            for p_ in range(NP_):
                nc.sync.dma_start(wo[p_][:], wo_d[p_ * P:(p_ + 1) * P, :])

            # V65 tiles padded to 128 cols/head (full-width LDWEIGHTS can be
            # pulled ahead of in-flight matmuls; 65-col ones serialize).
            # Cols 0-63 = V, col 64 = ones (softmax denominator), 65-127 = 0.
            VC = 128
            v65 = []
            for t in range(ST):
                v = pA.tile([P, H * VC], bf16, name=f"v65_{t}", tag="v65",
                            bufs=ST)
                nc.gpsimd.memset(v[:], 0.0)
                nc.gpsimd.memset(
                    v.rearrange("p (h c) -> p h c", c=VC)[:, :, 64:65], 1.0)
                v65.append(v)

            pExp_cm = tc.tile_pool(name="pExp", bufs=1)
            pExp = pExp_cm.__enter__()

            # ---- V projection (wv pool; chunks emitted inside pair 0) ----
            pV_cm = tc.tile_pool(name="pV", bufs=1)
            pV = pV_cm.__enter__()
            wv = []
            for d in range(DT):
                t = pV.tile([P, D], bf16, name=f"wv{d}", tag="wv", bufs=DT)
                nc.sync.dma_start(t[:], wv_d[d * P:(d + 1) * P, :])
                wv.append(t)
            hpn = ND // HD
            v_state = {}

            def emit_v_chunk(hc):
                """Half-chunk hc of the V projection (chain = hc//2)."""
                chain = hc // 2
                part = hc % 2
                t, n = chain // DL, chain % DL
                if part == 0:
                    v_state[chain] = psA.tile(
                        [P, ND], f32, name=f"vps{t}_{n}", tag="vqk", bufs=2)
                ps = v_state[chain]
                for d in range(4 * part, 4 * part + 4):
                    nc.tensor.matmul(
                        ps[:], xt[d][:, t * P:(t + 1) * P],
                        wv[d][:, n * ND:(n + 1) * ND],
                        start=(d == 0), stop=(d == DT - 1))
                if part == 1:
                    dst = v65[t].rearrange("p (h c) -> p h c", c=VC)[
                        :, n * hpn:(n + 1) * hpn, 0:64]
                    srcv = ps[:].rearrange("p (h k) -> p h k", k=HD)
                    bvs = bv_b[:, n * ND:(n + 1) * ND].rearrange(
                        "p (h k) -> p h k", k=HD)
                    nc.vector.tensor_add(dst, srcv, bvs)

            # ---- attention per head pair ----
            ctxT = [pCtx.tile([P, S], bf16, name=f"ctxT{p}", tag="ctxT",
                              bufs=NP_) for p in range(NP_)]

            def emit_qk_chain_part(p, chain, part, state):
                """Emit 4 of the 8 accumulation matmuls of QK chain
                (chain: 0..3 = Q-sl0, Q-sl1, K-sl0, K-sl1) for pair p."""
                wt, bc, dst = state["ops"][chain // 2]
                sl = chain % 2
                if part == 0:
                    state[chain] = psA.tile(
                        [P, NS], f32, name=f"qk{p}_{chain}", tag="vqk", bufs=2)
                ps = state[chain]
                for d in range(4 * part, 4 * part + 4):
                    nc.tensor.matmul(
                        ps[:], wt[:, d * P:(d + 1) * P],
                        xt[d][:, sl * NS:(sl + 1) * NS],
                        start=(d == 0), stop=(d == DT - 1))
                if part == 1:
                    nc.vector.tensor_scalar(
                        out=dst[:, sl * NS:(sl + 1) * NS], in0=ps[:],
                        scalar1=bc[:, p:p + 1], scalar2=None, op0=ALU.add)

            def make_qk_state(p):
                if p == 0:
                    wqt, wkt = wq0, wk0
                else:
                    wqt = pA.tile([P, DT * P], bf16, name=f"wq{p}", tag="wq",
                                  bufs=2)
                    nc.sync.dma_start(wqt[:], wq_d[p])
                    wkt = pA.tile([P, DT * P], bf16, name=f"wk{p}", tag="wk",
                                  bufs=2)
                    nc.sync.dma_start(wkt[:], wk_d[p])
                qt = pA.tile([P, S], bf16, name=f"qt{p}", tag="qt", bufs=2)
                kt = pA.tile([P, S], bf16, name=f"kt{p}", tag="kt", bufs=2)
                return {"ops": ((wqt, bqc, qt), (wkt, bkc, kt)),
                        "qt": qt, "kt": kt}

            LAG = 2
            qk_state = make_qk_state(0)
            for chain in range(4):
                for part in range(2):
                    emit_qk_chain_part(0, chain, part, qk_state)

            # deferred softmax-normalize queue: entries (p, sl, ctxU, den2)
            norm_q = []

            def emit_normalize():
                if not norm_q:
                    return
                p, sl, ctxU, den2 = norm_q.pop(0)
                den2f = pA.tile([2, NS], f32, name=f"d2f{p}_{sl}",
                                tag="den2f", bufs=3)
                nc.vector.reciprocal_approx_fast(den2f[:], den2[:])
                den2r = pA.tile([2, NS], f32r, name=f"d2r{p}_{sl}",
                                tag="den2r", bufs=3)
                nc.vector.tensor_copy(den2r[:], den2f[:])
                rcb = psA.tile([P, NS], f32, name=f"rcb{p}_{sl}",
                               tag="vqk", bufs=2)
                nc.tensor.matmul(rcb[:], sel2[:], den2r[:],
                                 start=True, stop=True)
                nc.vector.tensor_mul(
                    ctxT[p][:, sl * NS:(sl + 1) * NS],
                    ctxU[:, sl * NS:(sl + 1) * NS], rcb[:])

            pending = None
            for p in range(NP_):
                qt, kt = qk_state["qt"], qk_state["kt"]
                next_state = make_qk_state(p + 1) if p + 1 < NP_ else None

                ctxU = pA.tile([P, S], f32, name=f"ctxU{p}", tag="ctxU",
                               bufs=2)

                def emit_scores(sl, t, expt):
                    ps = psA.tile([P, 2 * NS], f32, name=f"sc{t}_{sl}",
                                  tag="sc", bufs=2)
                    for h in range(2):
                        nc.tensor.matmul(
                            ps[:, h * NS:(h + 1) * NS],
                            kt[h * HD:(h + 1) * HD, t * P:(t + 1) * P],
                            qt[h * HD:(h + 1) * HD, sl * NS:(sl + 1) * NS],
                            start=True, stop=True,
                            tile_position=(h * HD, 0))
                    e = pExp.tile([P, 2 * NS], bf16, name=f"e{t}_{sl}",
                                  tag="exp", bufs=3)
                    nc.scalar.activation(e[:], ps[:], AF.Exp, scale=scale)
                    expt[t] = e

                def emit_ctx(sl, tt, cps, expt):
                    for h in range(2):
                        hc = (2 * p + h) * VC
                        nc.tensor.matmul(
                            cps[h][:], v65[tt][:, hc:hc + VC],
                            expt[tt][:, h * NS:(h + 1) * NS],
                            start=(tt == 0), stop=(tt == ST - 1))

                def emit_evict(sl, cps):
                    den2 = pA.tile([2, NS], f32, name=f"den{p}_{sl}",
                                   tag="den2", bufs=3)
                    for h in range(2):
                        ps = cps[h]
                        stage = pA.tile([65, NS], f32, name=f"stg{h}{sl}",
                                        tag="rc", bufs=2)
                        nc.vector.tensor_copy(stage[64:65, :], ps[64:65, :])
                        nc.sync.dma_start(den2[h:h + 1, :], stage[64:65, :])
                        if h == 0:
                            nc.vector.tensor_copy(
                                ctxU[0:HD, sl * NS:(sl + 1) * NS],
                                ps[0:HD, :])
                        else:
                            tmp = pA.tile([HD, NS], f32, name=f"ctmp{sl}",
                                          tag="ctmp", bufs=2)
                            nc.vector.tensor_copy(tmp[:], ps[0:HD, :])
                            nc.sync.dma_start(
                                ctxU[HD:P, sl * NS:(sl + 1) * NS], tmp[:])
                    norm_q.append((p, sl, ctxU, den2))

                expt0 = {}
                cps0 = [psA.tile([P, NS], f32, name=f"cps{h}_0", tag="ctx",
                                 bufs=2) for h in range(2)]
                expt1 = {}
                cps1 = [psA.tile([P, NS], f32, name=f"cps{h}_1", tag="ctx",
                                 bufs=2) for h in range(2)]
                if p == 0:
                    # A: scores(sl0) + the whole V projection interleaved
                    for t in range(ST):
                        emit_scores(0, t, expt0)
                        for hc in range(4 * t, 4 * t + 4):
                            emit_v_chunk(hc)
                    # B: scores(sl1) + lagged ctx(sl0)
                    for t in range(ST + LAG):
                        if t < ST:
                            emit_scores(1, t, expt1)
                        if t >= LAG:
                            emit_ctx(0, t - LAG, cps0, expt0)
                    emit_evict(0, cps0)
                    # C: ctx(sl1) + QK(1) chunks
                    for t in range(ST):
                        emit_ctx(1, t, cps1, expt1)
                        if t == 2:
                            emit_normalize()          # (0, sl0)
                        if next_state is not None:
                            emit_qk_chain_part(p + 1, t // 2, t % 2,
                                               next_state)
                    emit_evict(1, cps1)
                    pV_cm.__exit__(None, None, None)
                else:
                    # A: scores(sl0) + QK(p+1) chunks 0-3 + lagged ctx(sl0)
                    for t in range(ST + LAG):
                        if t < ST:
                            emit_scores(0, t, expt0)
                            if next_state is not None and t < 4:
                                emit_qk_chain_part(p + 1, t // 2, t % 2,
                                                   next_state)
                        if t == 2:
                            emit_normalize()          # (p-1, sl1)
                        if t >= LAG:
                            emit_ctx(0, t - LAG, cps0, expt0)
                    emit_evict(0, cps0)
                    # B: scores(sl1) + QK(p+1) chunks 4-7 + lagged ctx(sl1)
                    for t in range(ST + LAG):
                        if t < ST:
                            emit_scores(1, t, expt1)
                            if next_state is not None and t < 4:
                                emit_qk_chain_part(p + 1, (t + 4) // 2,
                                                   t % 2, next_state)
                        if t == 2:
                            emit_normalize()          # (p, sl0)
                        if t >= LAG:
                            emit_ctx(1, t - LAG, cps1, expt1)
                    emit_evict(1, cps1)
                qk_state = next_state
            # pair 7, sl0 + sl1 normalize straight away (Wo accumulates
            # pair 7 last, giving the den chain ~3.5us of PE slack)
            emit_normalize()
            emit_normalize()
            pExp_cm.__exit__(None, None, None)

        # ---------------- resident pools (open after attention) ----------
        pH = octx.enter_context(tc.tile_pool(name="pH", bufs=1))
        ht = [pH.tile([P, S], bf16, name=f"ht{d}", tag="ht", bufs=DT)
              for d in range(DT)]
        h_nat = [pH.tile([P, D], bf16, name=f"hn{si}", tag="hn", bufs=ST)
                 for si in range(ST)]
        pW2 = octx.enter_context(tc.tile_pool(name="pW2", bufs=1))
        w2r = [pW2.tile([P, D], bf16, name=f"w2r{f}", tag="w2r", bufs=FT)
               for f in range(FT)]
        w1_tiles = []

        # ---------------- Wo + LN1 scope ----------------
        with tc.tile_pool(name="pWo", bufs=1) as pWo, \
             tc.tile_pool(name="psW", bufs=1, space="PSUM") as psW:
            g1_b = bcast_row(pWo, "g1", g1_d, D)
            be1_b = bcast_row(pWo, "be1", be1_d, D)
            # prefetch the first W1 stream tiles so FFN1 starts immediately
            for f in range(3):
                w1t = pW2.tile([P, DT * P], bf16, name=f"w1_{f}",
                               tag="w1", bufs=3)
                nc.sync.dma_start(w1t[:], w1_d[f])
                w1_tiles.append(w1t)

            def emit_tp(si):
                """h^T transposes for si (lagged 2 si so the LN1 chain
                latency never blocks the in-order PE queue); copybacks
                alternate DVE/ACT; then fold b2 into the residual."""
                hn = h_nat[si]
                for dd in range(DT):
                    ps = psW.tile([P, P], bf16, name=f"tp{si}_{dd}", tag="tp",
                                  bufs=4)
                    nc.tensor.transpose(
                        ps[:], hn[:, dd * P:(dd + 1) * P], identity[:])
                    dst = ht[dd][:, si * P:(si + 1) * P]
                    if dd % 2 == 0:
                        nc.vector.tensor_copy(dst, ps[:])
                    else:
                        nc.scalar.copy(dst, ps[:])

            for si in range(ST):
                xn = pWo.tile([P, D], f32, name=f"xn{si}", tag="xn", bufs=3)
                nc.sync.dma_start(xn[:], xbo_d[si * P:(si + 1) * P, :])
                pss = [psW.tile([P, ND], f32, name=f"c{si}_{n}", tag="c",
                                bufs=4) for n in range(DL)]
                for p in range(NP_):
                    for n in range(DL):
                        nc.tensor.matmul(
                            pss[n][:], ctxT[p][:, si * P:(si + 1) * P],
                            wo[p][:, n * ND:(n + 1) * ND],
                            start=(p == 0), stop=(p == NP_ - 1))
                v = pWo.tile([P, D], f32, name=f"v{si}", tag="v", bufs=3)
                for n in range(DL):
                    nc.vector.tensor_add(v[:, n * ND:(n + 1) * ND], pss[n][:],
                                         xn[:, n * ND:(n + 1) * ND])
                hn = h_nat[si]
                _layer_norm(nc, pWo, v, hn, g1_b, be1_b, si, "ln1")
                if si >= 3:
                    emit_tp(si - 3)
            for si in range(ST - 3, ST):
                emit_tp(si)

        # ---------------- FFN + LN2 scope (full sequence) ----------------
        with tc.tile_pool(name="pF", bufs=1) as pF:
            g2_b = bcast_row(pF, "g2", g2_d, D)
            be2_b = bcast_row(pF, "be2", be2_d, D)
            b2_b = bcast_row(pF, "b2", b2_d, D)

            # FFN1 + relu over full S; W1 streamed once (bf16)
            ut = []
            with tc.tile_pool(name="psU", bufs=1, space="PSUM") as psU:
                for f in range(FT):
                    if f < 3:
                        w1t = w1_tiles[f]
                    else:
                        w1t = pW2.tile([P, DT * P], bf16, name=f"w1_{f}",
                                       tag="w1", bufs=3)
                        nc.sync.dma_start(w1t[:], w1_d[f])
                    # W2 resident load rides along, spread over FFN1
                    nc.sync.dma_start(w2r[f][:], w2_d[f * P:(f + 1) * P, :])
                    u = pF.tile([P, S], bf16, name=f"ut{f}", tag="ut",
                                bufs=FT)
                    pss = [psU.tile([P, NS], f32, name=f"u{f}_{sh}", tag="u",
                                    bufs=4) for sh in range(SL)]
                    for d in range(DT):
                        for sh in range(SL):
                            nc.tensor.matmul(
                                pss[sh][:], w1t[:, d * P:(d + 1) * P],
                                ht[d][:, sh * NS:(sh + 1) * NS],
                                start=(d == 0), stop=(d == DT - 1))
                    for sh in range(SL):
                        nc.scalar.activation(
                            u[:, sh * NS:(sh + 1) * NS], pss[sh][:], AF.Relu,
                            bias=b1c[:, f:f + 1])
                    ut.append(u)
                    # fold the FFN2 output bias into the residual on the
                    # otherwise-idle DVE (transposes consumed clean h already)
                    if f < ST:
                        nc.vector.tensor_add(h_nat[f][:], h_nat[f][:],
                                             b2_b[:])

            # FFN2 + LN2 + out: per-si groups; W2 resident so the f-loop
            # restreams freely; LN2 of si overlaps matmuls of si+1
            with tc.tile_pool(name="psY", bufs=1, space="PSUM") as psY:
                for si in range(ST):
                    pss = [psY.tile([P, ND], f32, name=f"y{si}_{n}", tag="y",
                                    bufs=8) for n in range(DL)]
                    for f in range(FT):
                        for n in range(DL):
                            nc.tensor.matmul(
                                pss[n][:],
                                ut[f][:, si * P:(si + 1) * P],
                                w2r[f][:, n * ND:(n + 1) * ND],
                                start=(f == 0), stop=(f == FT - 1))
                    v = pF.tile([P, D], f32, name=f"v2_{si}", tag="v2",
                                bufs=3)
                    for n in range(DL):
                        nc.vector.tensor_add(
                            v[:, n * ND:(n + 1) * ND], pss[n][:],
                            h_nat[si][:, n * ND:(n + 1) * ND])
                    o = pF.tile([P, D], bf16, name=f"o{si}", tag="o",
                                bufs=3)
                    _layer_norm(nc, pF, v, o, g2_b, be2_b, si, "ln2")
                    nc.sync.dma_start(out_d[si * P:(si + 1) * P, :], o[:])

    nc.compile()
    return nc


from ml_dtypes import bfloat16 as _np_bf16


def pack_core_inputs(x_b, shared):
    """Per-core input map: batch element x_b + shared (prepacked) weights."""
    m = dict(shared)
    bo_row = m.pop("_bo_row")
    x_b = np.asarray(x_b, dtype=np.float32)
    m["xbo"] = np.ascontiguousarray(x_b + bo_row)
    m["xT"] = np.ascontiguousarray(x_b.T).astype(_np_bf16)
    return m


def pack_shared(Wq, bq, Wk, bk, Wv, bv, Wo, bo, ln1_g, ln1_b, W1, b1, W2, b2,
                ln2_g, ln2_b):
    """Host-side layout packing of the replicated weights (pure layout)."""
    from ml_dtypes import bfloat16 as np_bf16
    f = np.float32
    Wq = np.asarray(Wq, dtype=f); Wk = np.asarray(Wk, dtype=f)
    Wv = np.asarray(Wv, dtype=f)
    # [dt, dp, h2, q] -> [h2, dp, dt, q] contiguous
    pack_qk = lambda W: np.ascontiguousarray(
        W.reshape(D, H * HD).reshape(DT, P, NP_, P).transpose(2, 1, 0, 3)
        .reshape(NP_, P, DT * P))
    W1p = (np.asarray(W1, dtype=f).reshape(DT, P, FT, P)
           .transpose(2, 1, 0, 3).reshape(FT, P, DT * P))
    sel2 = np.zeros((2, P), dtype=f)
    sel2[0, 0:HD] = 1.0
    sel2[1, HD:P] = 1.0
    return {
        "sel2": sel2,
        "Wq": pack_qk(Wq).astype(np_bf16), "Wk": pack_qk(Wk).astype(np_bf16),
        "Wv": np.ascontiguousarray(Wv.reshape(D, D)).astype(np_bf16),
        "Wo": np.ascontiguousarray(Wo, dtype=f).astype(np_bf16),
        "W1": np.ascontiguousarray(W1p).astype(np_bf16),
        "W2": np.ascontiguousarray(W2, dtype=f).astype(np_bf16),
        "bqc": np.ascontiguousarray(np.asarray(bq, f).reshape(NP_, P).T),
        "bkc": np.ascontiguousarray(np.asarray(bk, f).reshape(NP_, P).T),
        "b1c": np.ascontiguousarray(np.asarray(b1, f).reshape(FT, P).T),
        "bv": np.ascontiguousarray(np.asarray(bv, f).reshape(D)),
        "b2": np.ascontiguousarray(b2, dtype=f),
        "g1": np.ascontiguousarray(ln1_g, dtype=f),
        "be1": np.ascontiguousarray(ln1_b, dtype=f),
        "g2": np.ascontiguousarray(ln2_g, dtype=f),
        "be2": np.ascontiguousarray(ln2_b, dtype=f),
        "_bo_row": np.asarray(bo, dtype=f).reshape(1, D),
    }


_NC_CACHE = {}


def get_nc():
    if "nc" not in _NC_CACHE:
        _NC_CACHE["nc"] = build_encoder(num_devices=8)
    return _NC_CACHE["nc"]


def kernel(x, Wq, bq, Wk, bk, Wv, bv, Wo, bo, ln1_g, ln1_b, W1, b1, W2, b2,
           ln2_g, ln2_b):
    x = np.asarray(x)
    assert x.shape == (B, S, D)
    shared = pack_shared(Wq, bq, Wk, bk, Wv, bv, Wo, bo, ln1_g, ln1_b,
                         W1, b1, W2, b2, ln2_g, ln2_b)
    in_maps = [pack_core_inputs(x[b], shared) for b in range(B)]
    nc = get_nc()
    res = bass_utils.run_bass_kernel_spmd(
        nc, in_maps, core_ids=list(range(B)), trace=False)
    return np.stack([np.asarray(res.results[b]["out"], dtype=np.float32)
                     for b in range(B)], axis=0)
